# revision 29
# baseline (speedup 1.0000x reference)
"""Trainium2 Bass kernel for nn_ExactAttention (block-diagonal sparse attention).

Reference computes dense softmax attention over [N,N] then masks to
block-diagonal segments (batch_seg is sorted).  Only the diagonal blocks
survive, so we compute segment-local attention only.

The reference subtracts the *global* max of Q@K^T before exp; softmax is
shift-invariant except through EPS=1e-8, whose effect is ~1e-8 relative
(denominators are O(100+)), far below the 2e-2 gate, so we skip the max
entirely (max |dot| ~ 70 -> exp(70/sqrt(128)) ~ 450, no overflow).

Sharding: segments are sorted by length (desc) and dealt round-robin:
slot j of every core gets one of ranks [8j, 8j+8), all padded to the
group max L_j, so all 8 cores run one SPMD program with near-zero
padding waste and balanced work.

Perf design (measured 37.1us baseline -> this):
  * rel-err budget is 2e-2; an all-fp16 pipeline measures 5.5e-4 on the
    real inputs, so every matmul runs at the PE's full 1 col/cycle rate:
    scores in ONE fp16 matmul (was 3 bf16 hi/lo passes), AV in fp16
    (was native fp32 = 4 col-cycles/col).
  * softmax denominator on the PE: den[1,L] += ones[ck,1]^T @ P_c.
    Zero-padded K rows give exp(0)=1 per padded key; the host subtracts
    (L - len) exactly.  Zero-padded V rows keep AV clean.
  * ACTIVATE costs (N+352)/1.2 ns and does NOT pipeline its fixed part,
    so exps are pair-merged: chunks land 512-aligned in a 2-bank PSUM
    tile and one EXP covers both (garbage cols are never read).
  * software pipelining: slot j+1's score matmuls are emitted BEFORE
    slot j's AV/den matmuls so the PE works while ACT runs exp(j).
  * q-halves on the sync HWDGE queue, k-halves + V on the scalar queue:
    two queues halve the first-data latency; outputs (den then O^T, per
    slot) go on sync.  No SWDGE use (its drains lengthen the epilogue).
  * slot order [2nd-largest, ..., largest, smallest]: the early slots
    eat the cold-clock window (HAM un-throttles after ~3.4us of PE
    activity; ~26 junk matmuls bridge the initial DMA wait), and the
    smallest slot's AV->cast->DMA chain ends the kernel.
  * outputs in fp16 (O^T) + fp32 (den); host divides and scatters.
"""

import numpy as np

import concourse.bass as bass
import concourse.mybir as mybir
import concourse.tile as tile
from concourse import bacc
from concourse import bass_utils

D = 128
N_CORES = 8
EPS = 1e-8
F32 = mybir.dt.float32
F16 = mybir.dt.float16
BF16 = mybir.dt.bfloat16

_program_cache = {}
_last_in_maps = None


def _build_program(slot_lens):
    """Build + compile the SPMD program for per-slot padded lengths."""
    key = tuple(slot_lens)
    if key in _program_cache:
        return _program_cache[key]

    scale = float(1.0 / np.sqrt(np.float32(D)))
    R = sum(slot_lens)
    offs = np.concatenate([[0], np.cumsum(slot_lens)]).astype(int)
    nkcs = [(L + 127) // 128 for L in slot_lens]
    choffs = np.concatenate([[0], np.cumsum(nkcs)]).astype(int)
    C = int(choffs[-1])

    nc = bacc.Bacc("TRN2", target_bir_lowering=False, debug=False,
                   num_devices=N_CORES)

    # q and k (fp16): slot j at cols offs[j]; each loads as ONE whole-tensor
    # DMA (per-partition lines ~3KB: full HWDGE rate, and every slot's
    # operands arrive together — no per-slot just-in-time stalls)
    q_d = nc.dram_tensor("q", [D, R], F16, kind="ExternalInput").ap()
    k_d = nc.dram_tensor("k", [D, R], F16, kind="ExternalInput").ap()
    # V chunk-major: [key-in-chunk, chunk*128 + d]
    vx_d = nc.dram_tensor("vx", [128, C * 128], F16, kind="ExternalInput").ap()
    o_d = nc.dram_tensor("o", [D, R], F16, kind="ExternalOutput").ap()
    # f16 den (values <=~1500, f16 max 65504; 2^-11 rel error is far under
    # the gate): halves the fragmented single-partition store
    den_d = nc.dram_tensor("den", [1, R], F16, kind="ExternalOutput").ap()

    with tile.TileContext(nc) as tc:
        with tc.tile_pool(name="qb", bufs=1) as q_pool, \
             tc.tile_pool(name="kb", bufs=1) as k_pool, \
             tc.tile_pool(name="v", bufs=1) as v_pool, \
             tc.tile_pool(name="p", bufs=5) as p_pool, \
             tc.tile_pool(name="ob", bufs=2) as ob_pool, \
             tc.tile_pool(name="dn", bufs=1) as dn_pool, \
             tc.tile_pool(name="one", bufs=1) as one_pool, \
             tc.tile_pool(name="wrm", bufs=1) as warm_pool, \
             tc.tile_pool(name="tps", bufs=2, space="PSUM") as t_psum, \
             tc.tile_pool(name="ops", bufs=2, space="PSUM") as o_psum, \
             tc.tile_pool(name="dps", bufs=2, space="PSUM") as d_psum:

            ones_sb = one_pool.tile([128, 1], F16)
            nc.vector.memset(ones_sb[:], 1.0)
            den_sb = dn_pool.tile([1, R], F16)

            # PE warm-up: HAM releases the clock throttle only after ~3.4us
            # of sustained PE activity; junk bf16 matmuls (on uninitialized
            # SBUF — results are discarded, the PSUM banks are overwritten
            # with start=True later) bridge the initial DMA wait so real
            # matmuls run at 2.4GHz as soon as data lands.
            wsb = warm_pool.tile([128, 128], BF16)
            nc.vector.memset(wsb[:], 0.0)
            for _ in range(2):
                wps = o_psum.tile([128, 128], F32, tag="o")
                for _ in range(8):
                    nc.tensor.matmul(wps[:], wsb[:, :128], wsb[:],
                                     start=True, stop=True)

            # input DMAs: q on sync, k + V on scalar (both HWDGE queues in
            # parallel), staggered to match consumption order: slot 0's q
            # and its first k chunk-pair lead (small, latency-bound — they
            # gate the first real matmuls at ~8.9us, which take over from
            # the junk warm-up), then the rest as big bursts.
            q_sb = q_pool.tile([D, R], F16)
            k_sb = k_pool.tile([D, R], F16)
            v_sb = v_pool.tile([128, C * 128], F16)
            L0 = slot_lens[0]
            kc = min(256, L0)
            nc.sync.dma_start(q_sb[:, :L0], q_d[:, :L0])
            nc.scalar.dma_start(k_sb[:, :kc], k_d[:, :kc])
            nc.scalar.dma_start(k_sb[:, kc:L0], k_d[:, kc:L0])
            nc.sync.dma_start(q_sb[:, L0:], q_d[:, L0:])
            nc.scalar.dma_start(k_sb[:, L0:], k_d[:, L0:])
            v_split = nkcs[0] * 128
            nc.scalar.dma_start(v_sb[:, :v_split], vx_d[:, :v_split])
            nc.scalar.dma_start(v_sb[:, v_split:], vx_d[:, v_split:])
            q_tiles = [q_sb[:, offs[j]:offs[j] + L]
                       for j, L in enumerate(slot_lens)]
            k_tiles = [k_sb[:, offs[j]:offs[j] + L]
                       for j, L in enumerate(slot_lens)]

            def emit_av_s(j, L, nkc, o0, c0, p_tiles, last=False):
                # AV chain first: the O^T cast + store (the long pole of the
                # output chain) starts as early as possible; the den (S)
                # chain and its DVE copy overlap the store.  All den rows
                # ship as ONE contiguous [1,R] DMA at the end — per-slot
                # 1-partition slices fragment into ~100B descriptors with
                # ~1.4us issue cost.
                o_ps = o_psum.tile([128, 512], F32, tag="o")
                for (c, ck, p_sb, m) in p_tiles:
                    nc.tensor.matmul(o_ps[:, :L],
                                     v_sb[:ck, (c0 + c) * 128:(c0 + c + 1) * 128],
                                     p_sb[:ck, m, :L],
                                     start=(c == 0), stop=(c == nkc - 1))
                d_ps = d_psum.tile([128, 512], F32, tag="d")
                for (c, ck, p_sb, m) in p_tiles:
                    nc.tensor.matmul(d_ps[:1, :L], ones_sb[:ck, :],
                                     p_sb[:ck, m, :L],
                                     start=(c == 0), stop=(c == nkc - 1))
                o_sb = ob_pool.tile([128, 512], F16, tag="ob")
                if last:
                    # final store in two halves: the first half's cast+DMA
                    # starts while the second half is still casting; both on
                    # sync (issues pipeline with the transfers), keeping the
                    # scalar queue free for the den store.
                    h = L // 2
                    nc.vector.tensor_copy(o_sb[:, :h], o_ps[:, :h])
                    nc.sync.dma_start(o_d[:, o0:o0 + h], o_sb[:, :h])
                    nc.vector.tensor_copy(o_sb[:, h:L], o_ps[:, h:L])
                    nc.sync.dma_start(o_d[:, o0 + h:o0 + L], o_sb[:, h:L])
                else:
                    nc.vector.tensor_copy(o_sb[:, :L], o_ps[:, :L])
                    nc.sync.dma_start(o_d[:, o0:o0 + L], o_sb[:, :L])
                nc.vector.tensor_copy(den_sb[:, o0:o0 + L], d_ps[:1, :L])

            pending = None
            for j, L in enumerate(slot_lens):
                nkc = nkcs[j]
                o0 = int(offs[j])
                c0 = int(choffs[j])
                q_ap = q_tiles[j]
                k_ap = k_tiles[j]
                p_tiles = []
                # chunk pairs share a 2-bank PSUM tile shaped [128, 2, 512]
                # (each matmul output stays inside one bank) and ONE merged
                # strided EXP covers exactly the [*, :, :L] valid columns.
                for c0p in range(0, nkc, 2):
                    npair = min(2, nkc - c0p)
                    t_ps = t_psum.tile([128, 2, 512], F32, tag="t")
                    p_sb = p_pool.tile([128, 2, 512], F16, tag="p")
                    for m in range(npair):
                        c = c0p + m
                        ck = min(128, L - c * 128)
                        nc.tensor.matmul(t_ps[:ck, m, :L],
                                         k_ap[:, c * 128:c * 128 + ck], q_ap[:],
                                         start=True, stop=True)
                        p_tiles.append((c, ck, p_sb, m))
                    if npair == 2:
                        nc.scalar.activation(p_sb[:, :, :L], t_ps[:, :, :L],
                                             mybir.ActivationFunctionType.Exp,
                                             scale=scale)
                    else:
                        nc.scalar.activation(p_sb[:, 0, :L], t_ps[:, 0, :L],
                                             mybir.ActivationFunctionType.Exp,
                                             scale=scale)
                if pending is not None:
                    emit_av_s(*pending)
                pending = (j, L, nkc, o0, c0, p_tiles)
            emit_av_s(*pending, last=True)
            # one contiguous den store (single descriptor) on the scalar
            # queue, free after the last exp
            nc.scalar.dma_start(den_d[:], den_sb[:])

    nc.compile()
    _program_cache[key] = nc
    return nc


def _reference_host(Q, K, V, num_batch, batch_seg):
    """Pure-NumPy fallback for input shapes outside the tuned envelope."""
    dot = Q.astype(np.float64) @ K.T.astype(np.float64)
    A = np.exp((dot - dot.max()) / np.sqrt(np.float64(Q.shape[-1])))
    if num_batch > 1:
        A = np.where(batch_seg[None, :] == batch_seg[:, None], A, 0.0)
    return ((A / (A.sum(-1, keepdims=True) + EPS)) @ V.astype(np.float64)
            ).astype(np.float32)


def kernel(Q, K, V, num_batch, batch_seg):
    Q = np.asarray(Q, dtype=np.float32)
    K = np.asarray(K, dtype=np.float32)
    V = np.asarray(V, dtype=np.float32)
    batch_seg = np.asarray(batch_seg)
    N = Q.shape[0]
    nb = int(num_batch)

    counts = np.bincount(batch_seg.astype(np.int64), minlength=max(nb, 1))
    if nb < 2 or (counts.max() if nb else N) > 512:
        return _reference_host(Q, K, V, nb, batch_seg)

    # row indices per segment (robust to unsorted batch_seg)
    row_order = np.argsort(batch_seg, kind="stable")
    starts = np.zeros(nb + 1, dtype=np.int64)
    np.cumsum(counts, out=starts[1:])

    # rank segments by length desc, group into slots of 8.  Slot order:
    # ascending by group max, except the largest group is placed 3rd and
    # the smallest LAST — early slots eat the cold-clock window, and the
    # smallest slot's output chain ends the kernel.
    order = np.argsort(-counts, kind="stable")
    n_slots = (nb + N_CORES - 1) // N_CORES
    groups = []  # (Lmax, members) ascending by Lmax
    for j in range(n_slots):
        grp = order[(n_slots - 1 - j) * N_CORES:(n_slots - j) * N_CORES]
        groups.append((max(1, int(counts[grp].max())), grp))
    perm = list(range(n_slots))
    if n_slots >= 2:
        perm = perm[1:] + perm[:1]  # smallest last, 2nd-smallest first
    slot_lens = []
    assign = {}  # (core, slot) -> seg id
    for j, gi in enumerate(perm):
        Lmax, grp = groups[gi]
        slot_lens.append(Lmax)
        for c, seg in enumerate(grp):
            assign[(c, j)] = int(seg)

    offs = np.concatenate([[0], np.cumsum(slot_lens)]).astype(int)
    nkcs = [(L + 127) // 128 for L in slot_lens]
    choffs = np.concatenate([[0], np.cumsum(nkcs)]).astype(int)
    R = int(offs[-1])
    C = int(choffs[-1])

    nc = _build_program(tuple(slot_lens))

    in_maps = []
    for core in range(N_CORES):
        Qp = np.zeros((R, D), np.float32)
        Kp = np.zeros((R, D), np.float32)
        Vp = np.zeros((C * 128, D), np.float32)
        for j in range(n_slots):
            seg = assign.get((core, j))
            if seg is None:
                continue
            b0, b1 = starts[seg], starts[seg + 1]
            ln = int(b1 - b0)
            if ln == 0:
                continue
            ridx = row_order[b0:b1]
            o0 = int(offs[j])
            Qp[o0:o0 + ln] = Q[ridx]
            Kp[o0:o0 + ln] = K[ridx]
            v0 = int(choffs[j]) * 128
            Vp[v0:v0 + ln] = V[ridx]
        vh = np.ascontiguousarray(
            Vp.reshape(C, 128, D).transpose(1, 0, 2)
        ).reshape(128, C * D).astype(np.float16)
        in_maps.append({
            "q": np.ascontiguousarray(Qp.T).astype(np.float16),
            "k": np.ascontiguousarray(Kp.T).astype(np.float16),
            "vx": vh,
        })

    global _last_in_maps
    _last_in_maps = in_maps
    res = bass_utils.run_bass_kernel_spmd(nc, in_maps,
                                          core_ids=list(range(N_CORES)))

    out = np.empty((N, D), np.float32)
    for (core, j), seg in assign.items():
        b0, b1 = starts[seg], starts[seg + 1]
        ln = int(b1 - b0)
        if ln == 0:
            continue
        o0 = int(offs[j])
        L = slot_lens[j]
        otT = res.results[core]["o"][:, o0:o0 + ln].astype(np.float32)
        den_raw = res.results[core]["den"][0, o0:o0 + ln].astype(np.float64)
        # padded keys (zero K) contribute exp(0)=1 each to the device den
        den = den_raw - float(L - ln) + EPS
        out[row_order[b0:b1]] = (otT / den[None, :]).T.astype(np.float32)
    return out


# revision 30
# speedup vs baseline: 1.0396x; 1.0396x over previous
"""Trainium2 Bass kernel for nn_ExactAttention (block-diagonal sparse attention).

Reference computes dense softmax attention over [N,N] then masks to
block-diagonal segments (batch_seg is sorted).  Only the diagonal blocks
survive, so we compute segment-local attention only.

The reference subtracts the *global* max of Q@K^T before exp; softmax is
shift-invariant except through EPS=1e-8, whose effect is ~1e-8 relative
(denominators are O(100+)), far below the 2e-2 gate, so we skip the max
entirely (max |dot| ~ 70 -> exp(70/sqrt(128)) ~ 450, no overflow).

Sharding: segments are sorted by length (desc) and dealt round-robin:
slot j of every core gets one of ranks [8j, 8j+8), all padded to the
group max L_j, so all 8 cores run one SPMD program with near-zero
padding waste and balanced work.

Perf design (measured 37.1us baseline -> this):
  * rel-err budget is 2e-2; an all-fp16 pipeline measures 5.5e-4 on the
    real inputs, so every matmul runs at the PE's full 1 col/cycle rate:
    scores in ONE fp16 matmul (was 3 bf16 hi/lo passes), AV in fp16
    (was native fp32 = 4 col-cycles/col).
  * softmax denominator on the PE: den[1,L] += ones[ck,1]^T @ P_c.
    Zero-padded K rows give exp(0)=1 per padded key; the host subtracts
    (L - len) exactly.  Zero-padded V rows keep AV clean.
  * ACTIVATE costs (N+352)/1.2 ns and does NOT pipeline its fixed part,
    so exps are pair-merged: chunks land 512-aligned in a 2-bank PSUM
    tile and one EXP covers both (garbage cols are never read).
  * software pipelining: slot j+1's score matmuls are emitted BEFORE
    slot j's AV/den matmuls so the PE works while ACT runs exp(j).
  * q-halves on the sync HWDGE queue, k-halves + V on the scalar queue:
    two queues halve the first-data latency; outputs (den then O^T, per
    slot) go on sync.  No SWDGE use (its drains lengthen the epilogue).
  * slot order [2nd-largest, ..., largest, smallest]: the early slots
    eat the cold-clock window (HAM un-throttles after ~3.4us of PE
    activity; ~26 junk matmuls bridge the initial DMA wait), and the
    smallest slot's AV->cast->DMA chain ends the kernel.
  * outputs in fp16 (O^T) + fp32 (den); host divides and scatters.
"""

import numpy as np

import concourse.bass as bass
import concourse.mybir as mybir
import concourse.tile as tile
from concourse import bacc
from concourse import bass_utils

D = 128
N_CORES = 8
EPS = 1e-8
F32 = mybir.dt.float32
F16 = mybir.dt.float16
BF16 = mybir.dt.bfloat16

_program_cache = {}
_last_in_maps = None


def _build_program(slot_lens):
    """Build + compile the SPMD program for per-slot padded lengths."""
    key = tuple(slot_lens)
    if key in _program_cache:
        return _program_cache[key]

    scale = float(1.0 / np.sqrt(np.float32(D)))
    R = sum(slot_lens)
    offs = np.concatenate([[0], np.cumsum(slot_lens)]).astype(int)
    nkcs = [(L + 127) // 128 for L in slot_lens]
    choffs = np.concatenate([[0], np.cumsum(nkcs)]).astype(int)
    C = int(choffs[-1])

    nc = bacc.Bacc("TRN2", target_bir_lowering=False, debug=False,
                   num_devices=N_CORES)

    # q and k (fp16): slot j at cols offs[j]; each loads as ONE whole-tensor
    # DMA (per-partition lines ~3KB: full HWDGE rate, and every slot's
    # operands arrive together — no per-slot just-in-time stalls)
    q_d = nc.dram_tensor("q", [D, R], F16, kind="ExternalInput").ap()
    k_d = nc.dram_tensor("k", [D, R], F16, kind="ExternalInput").ap()
    # V chunk-major: [key-in-chunk, chunk*128 + d]
    vx_d = nc.dram_tensor("vx", [128, C * 128], F16, kind="ExternalInput").ap()
    o_d = nc.dram_tensor("o", [D, R], F16, kind="ExternalOutput").ap()
    # f16 den (values <=~1500, f16 max 65504; 2^-11 rel error is far under
    # the gate): halves the fragmented single-partition store
    den_d = nc.dram_tensor("den", [1, R], F16, kind="ExternalOutput").ap()

    with tile.TileContext(nc) as tc:
        with tc.tile_pool(name="qb", bufs=1) as q_pool, \
             tc.tile_pool(name="kb", bufs=1) as k_pool, \
             tc.tile_pool(name="v", bufs=1) as v_pool, \
             tc.tile_pool(name="p", bufs=5) as p_pool, \
             tc.tile_pool(name="ob", bufs=2) as ob_pool, \
             tc.tile_pool(name="dn", bufs=1) as dn_pool, \
             tc.tile_pool(name="one", bufs=1) as one_pool, \
             tc.tile_pool(name="wrm", bufs=1) as warm_pool, \
             tc.tile_pool(name="tps", bufs=2, space="PSUM") as t_psum, \
             tc.tile_pool(name="ops", bufs=2, space="PSUM") as o_psum, \
             tc.tile_pool(name="dps", bufs=2, space="PSUM") as d_psum:

            ones_sb = one_pool.tile([128, 1], F16)
            nc.vector.memset(ones_sb[:], 1.0)
            den_sb = dn_pool.tile([1, R], F16)

            # PE warm-up: HAM releases the clock throttle only after ~3.4us
            # of sustained PE activity; junk bf16 matmuls (on uninitialized
            # SBUF — results are discarded, the PSUM banks are overwritten
            # with start=True later) bridge the initial DMA wait so real
            # matmuls run at 2.4GHz as soon as data lands.
            wsb = warm_pool.tile([128, 128], BF16)
            nc.vector.memset(wsb[:], 0.0)
            for _ in range(2):
                wps = o_psum.tile([128, 128], F32, tag="o")
                for _ in range(15):
                    nc.tensor.matmul(wps[:], wsb[:, :128], wsb[:],
                                     start=True, stop=True)

            # input DMAs: q on sync, k + V on scalar (both HWDGE queues in
            # parallel), staggered to match consumption order: slot 0,
            # slot 1, then the rest.  The junk warm-up bridges until slot
            # 0's data is fully resident — starting real matmuls earlier
            # makes them stall mid-stream, which resets the HAM activity
            # window and leaves the clock throttled (measured).
            q_sb = q_pool.tile([D, R], F16)
            k_sb = k_pool.tile([D, R], F16)
            v_sb = v_pool.tile([128, C * 128], F16)
            cuts = [0]
            for j in range(min(2, len(slot_lens))):
                cuts.append(int(offs[j + 1]))
            cuts.append(R)
            for a, b in zip(cuts[:-1], cuts[1:]):
                if a < b:
                    nc.sync.dma_start(q_sb[:, a:b], q_d[:, a:b])
                    nc.scalar.dma_start(k_sb[:, a:b], k_d[:, a:b])
            v_split = nkcs[0] * 128
            nc.scalar.dma_start(v_sb[:, :v_split], vx_d[:, :v_split])
            nc.scalar.dma_start(v_sb[:, v_split:], vx_d[:, v_split:])
            q_tiles = [q_sb[:, offs[j]:offs[j] + L]
                       for j, L in enumerate(slot_lens)]
            k_tiles = [k_sb[:, offs[j]:offs[j] + L]
                       for j, L in enumerate(slot_lens)]

            def emit_av_s(j, L, nkc, o0, c0, p_tiles, last=False):
                # AV chain first: the O^T cast + store (the long pole of the
                # output chain) starts as early as possible; the den (S)
                # chain and its DVE copy overlap the store.  All den rows
                # ship as ONE contiguous [1,R] DMA at the end — per-slot
                # 1-partition slices fragment into ~100B descriptors with
                # ~1.4us issue cost.
                o_ps = o_psum.tile([128, 512], F32, tag="o")
                for (c, ck, p_sb, m) in p_tiles:
                    nc.tensor.matmul(o_ps[:, :L],
                                     v_sb[:ck, (c0 + c) * 128:(c0 + c + 1) * 128],
                                     p_sb[:ck, m, :L],
                                     start=(c == 0), stop=(c == nkc - 1))
                d_ps = d_psum.tile([128, 512], F32, tag="d")
                for (c, ck, p_sb, m) in p_tiles:
                    nc.tensor.matmul(d_ps[:1, :L], ones_sb[:ck, :],
                                     p_sb[:ck, m, :L],
                                     start=(c == 0), stop=(c == nkc - 1))
                o_sb = ob_pool.tile([128, 512], F16, tag="ob")
                if last:
                    # final store in two halves: the first half's cast+DMA
                    # starts while the second half is still casting; both on
                    # sync (issues pipeline with the transfers), keeping the
                    # scalar queue free for the den store.
                    h = L // 2
                    nc.vector.tensor_copy(o_sb[:, :h], o_ps[:, :h])
                    nc.sync.dma_start(o_d[:, o0:o0 + h], o_sb[:, :h])
                    nc.vector.tensor_copy(o_sb[:, h:L], o_ps[:, h:L])
                    nc.sync.dma_start(o_d[:, o0 + h:o0 + L], o_sb[:, h:L])
                else:
                    nc.vector.tensor_copy(o_sb[:, :L], o_ps[:, :L])
                    nc.sync.dma_start(o_d[:, o0:o0 + L], o_sb[:, :L])
                nc.vector.tensor_copy(den_sb[:, o0:o0 + L], d_ps[:1, :L])

            pending = None
            for j, L in enumerate(slot_lens):
                nkc = nkcs[j]
                o0 = int(offs[j])
                c0 = int(choffs[j])
                q_ap = q_tiles[j]
                k_ap = k_tiles[j]
                p_tiles = []
                # chunk pairs share a 2-bank PSUM tile shaped [128, 2, 512]
                # (each matmul output stays inside one bank) and ONE merged
                # strided EXP covers exactly the [*, :, :L] valid columns.
                for c0p in range(0, nkc, 2):
                    npair = min(2, nkc - c0p)
                    t_ps = t_psum.tile([128, 2, 512], F32, tag="t")
                    p_sb = p_pool.tile([128, 2, 512], F16, tag="p")
                    for m in range(npair):
                        c = c0p + m
                        ck = min(128, L - c * 128)
                        nc.tensor.matmul(t_ps[:ck, m, :L],
                                         k_ap[:, c * 128:c * 128 + ck], q_ap[:],
                                         start=True, stop=True)
                        p_tiles.append((c, ck, p_sb, m))
                    if npair == 2:
                        nc.scalar.activation(p_sb[:, :, :L], t_ps[:, :, :L],
                                             mybir.ActivationFunctionType.Exp,
                                             scale=scale)
                    else:
                        nc.scalar.activation(p_sb[:, 0, :L], t_ps[:, 0, :L],
                                             mybir.ActivationFunctionType.Exp,
                                             scale=scale)
                if pending is not None:
                    emit_av_s(*pending)
                pending = (j, L, nkc, o0, c0, p_tiles)
            emit_av_s(*pending, last=True)
            # one contiguous den store (single descriptor) on the scalar
            # queue, free after the last exp
            nc.scalar.dma_start(den_d[:], den_sb[:])

    nc.compile()
    _program_cache[key] = nc
    return nc


def _reference_host(Q, K, V, num_batch, batch_seg):
    """Pure-NumPy fallback for input shapes outside the tuned envelope."""
    dot = Q.astype(np.float64) @ K.T.astype(np.float64)
    A = np.exp((dot - dot.max()) / np.sqrt(np.float64(Q.shape[-1])))
    if num_batch > 1:
        A = np.where(batch_seg[None, :] == batch_seg[:, None], A, 0.0)
    return ((A / (A.sum(-1, keepdims=True) + EPS)) @ V.astype(np.float64)
            ).astype(np.float32)


def kernel(Q, K, V, num_batch, batch_seg):
    Q = np.asarray(Q, dtype=np.float32)
    K = np.asarray(K, dtype=np.float32)
    V = np.asarray(V, dtype=np.float32)
    batch_seg = np.asarray(batch_seg)
    N = Q.shape[0]
    nb = int(num_batch)

    counts = np.bincount(batch_seg.astype(np.int64), minlength=max(nb, 1))
    if nb < 2 or (counts.max() if nb else N) > 512:
        return _reference_host(Q, K, V, nb, batch_seg)

    # row indices per segment (robust to unsorted batch_seg)
    row_order = np.argsort(batch_seg, kind="stable")
    starts = np.zeros(nb + 1, dtype=np.int64)
    np.cumsum(counts, out=starts[1:])

    # rank segments by length desc, group into slots of 8.  Slot order:
    # ascending by group max, except the largest group is placed 3rd and
    # the smallest LAST — early slots eat the cold-clock window, and the
    # smallest slot's output chain ends the kernel.
    order = np.argsort(-counts, kind="stable")
    n_slots = (nb + N_CORES - 1) // N_CORES
    groups = []  # (Lmax, members) ascending by Lmax
    for j in range(n_slots):
        grp = order[(n_slots - 1 - j) * N_CORES:(n_slots - j) * N_CORES]
        groups.append((max(1, int(counts[grp].max())), grp))
    perm = list(range(n_slots))
    if n_slots >= 2:
        perm = perm[1:] + perm[:1]  # smallest last, 2nd-smallest first
    slot_lens = []
    assign = {}  # (core, slot) -> seg id
    for j, gi in enumerate(perm):
        Lmax, grp = groups[gi]
        slot_lens.append(Lmax)
        for c, seg in enumerate(grp):
            assign[(c, j)] = int(seg)

    offs = np.concatenate([[0], np.cumsum(slot_lens)]).astype(int)
    nkcs = [(L + 127) // 128 for L in slot_lens]
    choffs = np.concatenate([[0], np.cumsum(nkcs)]).astype(int)
    R = int(offs[-1])
    C = int(choffs[-1])

    nc = _build_program(tuple(slot_lens))

    in_maps = []
    for core in range(N_CORES):
        Qp = np.zeros((R, D), np.float32)
        Kp = np.zeros((R, D), np.float32)
        Vp = np.zeros((C * 128, D), np.float32)
        for j in range(n_slots):
            seg = assign.get((core, j))
            if seg is None:
                continue
            b0, b1 = starts[seg], starts[seg + 1]
            ln = int(b1 - b0)
            if ln == 0:
                continue
            ridx = row_order[b0:b1]
            o0 = int(offs[j])
            Qp[o0:o0 + ln] = Q[ridx]
            Kp[o0:o0 + ln] = K[ridx]
            v0 = int(choffs[j]) * 128
            Vp[v0:v0 + ln] = V[ridx]
        vh = np.ascontiguousarray(
            Vp.reshape(C, 128, D).transpose(1, 0, 2)
        ).reshape(128, C * D).astype(np.float16)
        in_maps.append({
            "q": np.ascontiguousarray(Qp.T).astype(np.float16),
            "k": np.ascontiguousarray(Kp.T).astype(np.float16),
            "vx": vh,
        })

    global _last_in_maps
    _last_in_maps = in_maps
    res = bass_utils.run_bass_kernel_spmd(nc, in_maps,
                                          core_ids=list(range(N_CORES)))

    out = np.empty((N, D), np.float32)
    for (core, j), seg in assign.items():
        b0, b1 = starts[seg], starts[seg + 1]
        ln = int(b1 - b0)
        if ln == 0:
            continue
        o0 = int(offs[j])
        L = slot_lens[j]
        otT = res.results[core]["o"][:, o0:o0 + ln].astype(np.float32)
        den_raw = res.results[core]["den"][0, o0:o0 + ln].astype(np.float64)
        # padded keys (zero K) contribute exp(0)=1 each to the device den
        den = den_raw - float(L - ln) + EPS
        out[row_order[b0:b1]] = (otT / den[None, :]).T.astype(np.float32)
    return out


# revision 33
# speedup vs baseline: 1.0647x; 1.0242x over previous
"""Trainium2 Bass kernel for nn_ExactAttention (block-diagonal sparse attention).

Reference computes dense softmax attention over [N,N] then masks to
block-diagonal segments (batch_seg is sorted).  Only the diagonal blocks
survive, so we compute segment-local attention only.

The reference subtracts the *global* max of Q@K^T before exp; softmax is
shift-invariant except through EPS=1e-8, whose effect is ~1e-8 relative
(denominators are O(100+)), far below the 2e-2 gate, so we skip the max
entirely (max |dot| ~ 70 -> exp(70/sqrt(128)) ~ 450, no overflow).

Sharding: segments are sorted by length (desc) and dealt round-robin:
slot j of every core gets one of ranks [8j, 8j+8), all padded to the
group max L_j, so all 8 cores run one SPMD program with near-zero
padding waste and balanced work.

Perf design (measured 37.1us baseline -> this):
  * rel-err budget is 2e-2; an all-fp16 pipeline measures 5.5e-4 on the
    real inputs, so every matmul runs at the PE's full 1 col/cycle rate:
    scores in ONE fp16 matmul (was 3 bf16 hi/lo passes), AV in fp16
    (was native fp32 = 4 col-cycles/col).
  * softmax denominator on the PE: den[1,L] += ones[ck,1]^T @ P_c.
    Zero-padded K rows give exp(0)=1 per padded key; the host subtracts
    (L - len) exactly.  Zero-padded V rows keep AV clean.
  * ACTIVATE costs (N+352)/1.2 ns and does NOT pipeline its fixed part,
    so exps are pair-merged: chunks land in a [128,2,512] 2-bank PSUM
    tile (a matmul output must stay inside one bank) and one strided EXP
    covers exactly the [*,:,:L] valid columns of both chunks.
  * software pipelining: slot j+1's score matmuls are emitted BEFORE
    slot j's AV/den matmuls so the PE works while ACT runs exp(j).
  * q on the sync HWDGE queue, k + V on the scalar queue, staggered
    slot0/slot1/rest; O^T stores on sync per slot; den ships as ONE
    contiguous [1,R] f16 store at the end (per-slot single-partition
    slices fragment into ~100B descriptors with ~1.4us issue cost).
    No SWDGE use (its drains lengthen the epilogue).
  * slot order [2nd-largest, ..., largest, smallest]: the early slots
    eat the cold-clock window (HAM un-throttles after ~3.4us of PE
    activity; ~30 junk matmuls bridge the initial DMA wait — real
    matmuls must NOT start before their data is fully resident, or
    mid-stream stalls reset the HAM window and the clock stays cold),
    and the smallest slot's AV->cast->DMA chain ends the kernel.
  * outputs in fp16 (O^T and den); host divides and scatters.
"""

import numpy as np

import concourse.bass as bass
import concourse.mybir as mybir
import concourse.tile as tile
from concourse import bacc
from concourse import bass_utils

D = 128
N_CORES = 8
EPS = 1e-8
F32 = mybir.dt.float32
F16 = mybir.dt.float16
BF16 = mybir.dt.bfloat16

_program_cache = {}
_last_in_maps = None


def _build_program(slot_lens):
    """Build + compile the SPMD program for per-slot padded lengths."""
    key = tuple(slot_lens)
    if key in _program_cache:
        return _program_cache[key]

    scale = float(1.0 / np.sqrt(np.float32(D)))
    R = sum(slot_lens)
    offs = np.concatenate([[0], np.cumsum(slot_lens)]).astype(int)
    nkcs = [(L + 127) // 128 for L in slot_lens]
    choffs = np.concatenate([[0], np.cumsum(nkcs)]).astype(int)
    C = int(choffs[-1])

    nc = bacc.Bacc("TRN2", target_bir_lowering=False, debug=False,
                   num_devices=N_CORES)

    # q and k (fp16): slot j at cols offs[j]; each loads as ONE whole-tensor
    # DMA (per-partition lines ~3KB: full HWDGE rate, and every slot's
    # operands arrive together — no per-slot just-in-time stalls)
    q_d = nc.dram_tensor("q", [D, R], F16, kind="ExternalInput").ap()
    k_d = nc.dram_tensor("k", [D, R], F16, kind="ExternalInput").ap()
    # V chunk-major: [key-in-chunk, chunk*128 + d]
    vx_d = nc.dram_tensor("vx", [128, C * 128], F16, kind="ExternalInput").ap()
    o_d = nc.dram_tensor("o", [D, R], F16, kind="ExternalOutput").ap()
    # f16 den (values <=~1500, f16 max 65504; 2^-11 rel error is far under
    # the gate): halves the fragmented single-partition store
    den_d = nc.dram_tensor("den", [1, R], F16, kind="ExternalOutput").ap()

    with tile.TileContext(nc) as tc:
        with tc.tile_pool(name="qb", bufs=1) as q_pool, \
             tc.tile_pool(name="kb", bufs=1) as k_pool, \
             tc.tile_pool(name="v", bufs=1) as v_pool, \
             tc.tile_pool(name="p", bufs=5) as p_pool, \
             tc.tile_pool(name="ob", bufs=2) as ob_pool, \
             tc.tile_pool(name="dn", bufs=1) as dn_pool, \
             tc.tile_pool(name="one", bufs=1) as one_pool, \
             tc.tile_pool(name="wrm", bufs=1) as warm_pool, \
             tc.tile_pool(name="tps", bufs=2, space="PSUM") as t_psum, \
             tc.tile_pool(name="ops", bufs=2, space="PSUM") as o_psum, \
             tc.tile_pool(name="dps", bufs=2, space="PSUM") as d_psum:

            ones_sb = one_pool.tile([128, 1], F16)
            nc.vector.memset(ones_sb[:], 1.0)
            den_sb = dn_pool.tile([1, R], F16)

            # PE warm-up: HAM releases the clock throttle only after ~3.4us
            # of sustained PE activity; junk bf16 matmuls (on uninitialized
            # SBUF — results are discarded, the PSUM banks are overwritten
            # with start=True later) bridge the initial DMA wait so real
            # matmuls run at 2.4GHz as soon as data lands.
            wsb = warm_pool.tile([128, 128], BF16)
            nc.vector.memset(wsb[:], 0.0)
            for _ in range(2):
                wps = o_psum.tile([128, 128], F32, tag="o")
                for _ in range(14):
                    nc.tensor.matmul(wps[:], wsb[:, :128], wsb[:],
                                     start=True, stop=True)

            # input DMAs: q on sync, k + V on scalar (both HWDGE queues in
            # parallel), staggered to match consumption order: slot 0,
            # slot 1, then the rest.  The junk warm-up bridges until slot
            # 0's data is fully resident — starting real matmuls earlier
            # makes them stall mid-stream, which resets the HAM activity
            # window and leaves the clock throttled (measured).
            q_sb = q_pool.tile([D, R], F16)
            k_sb = k_pool.tile([D, R], F16)
            v_sb = v_pool.tile([128, C * 128], F16)
            cuts = [0]
            for j in range(min(2, len(slot_lens))):
                cuts.append(int(offs[j + 1]))
            cuts.append(R)
            for a, b in zip(cuts[:-1], cuts[1:]):
                if a < b:
                    nc.sync.dma_start(q_sb[:, a:b], q_d[:, a:b])
                    nc.scalar.dma_start(k_sb[:, a:b], k_d[:, a:b])
            v_split = nkcs[0] * 128
            nc.scalar.dma_start(v_sb[:, :v_split], vx_d[:, :v_split])
            nc.scalar.dma_start(v_sb[:, v_split:], vx_d[:, v_split:])
            q_tiles = [q_sb[:, offs[j]:offs[j] + L]
                       for j, L in enumerate(slot_lens)]
            k_tiles = [k_sb[:, offs[j]:offs[j] + L]
                       for j, L in enumerate(slot_lens)]

            def emit_av_s(j, L, nkc, o0, c0, p_tiles, last=False):
                # AV chain first: the O^T cast + store (the long pole of the
                # output chain) starts as early as possible; the den (S)
                # chain and its DVE copy overlap the store.  All den rows
                # ship as ONE contiguous [1,R] DMA at the end — per-slot
                # 1-partition slices fragment into ~100B descriptors with
                # ~1.4us issue cost.
                o_ps = o_psum.tile([128, 512], F32, tag="o")
                for (c, ck, p_sb, m) in p_tiles:
                    nc.tensor.matmul(o_ps[:, :L],
                                     v_sb[:ck, (c0 + c) * 128:(c0 + c + 1) * 128],
                                     p_sb[:ck, m, :L],
                                     start=(c == 0), stop=(c == nkc - 1))
                d_ps = d_psum.tile([128, 512], F32, tag="d")
                for (c, ck, p_sb, m) in p_tiles:
                    nc.tensor.matmul(d_ps[:1, :L], ones_sb[:ck, :],
                                     p_sb[:ck, m, :L],
                                     start=(c == 0), stop=(c == nkc - 1))
                o_sb = ob_pool.tile([128, 512], F16, tag="ob")
                if last:
                    # final store in two halves: the first half's cast+DMA
                    # starts while the second half is still casting; both on
                    # sync (issues pipeline with the transfers).  The den
                    # copy runs on ACT (idle after the last exp) so it and
                    # its scalar-queue store never queue behind the O casts
                    # on DVE — the last DMA completion gates the teardown.
                    nc.scalar.copy(den_sb[:, o0:o0 + L], d_ps[:1, :L])
                    h = L // 2
                    nc.vector.tensor_copy(o_sb[:, :h], o_ps[:, :h])
                    nc.sync.dma_start(o_d[:, o0:o0 + h], o_sb[:, :h])
                    nc.vector.tensor_copy(o_sb[:, h:L], o_ps[:, h:L])
                    nc.sync.dma_start(o_d[:, o0 + h:o0 + L], o_sb[:, h:L])
                else:
                    nc.vector.tensor_copy(o_sb[:, :L], o_ps[:, :L])
                    nc.sync.dma_start(o_d[:, o0:o0 + L], o_sb[:, :L])
                    nc.vector.tensor_copy(den_sb[:, o0:o0 + L], d_ps[:1, :L])

            pending = None
            for j, L in enumerate(slot_lens):
                nkc = nkcs[j]
                o0 = int(offs[j])
                c0 = int(choffs[j])
                q_ap = q_tiles[j]
                k_ap = k_tiles[j]
                p_tiles = []
                # chunk pairs share a 2-bank PSUM tile shaped [128, 2, 512]
                # (each matmul output stays inside one bank) and ONE merged
                # strided EXP covers exactly the [*, :, :L] valid columns.
                for c0p in range(0, nkc, 2):
                    npair = min(2, nkc - c0p)
                    t_ps = t_psum.tile([128, 2, 512], F32, tag="t")
                    p_sb = p_pool.tile([128, 2, 512], F16, tag="p")
                    for m in range(npair):
                        c = c0p + m
                        ck = min(128, L - c * 128)
                        nc.tensor.matmul(t_ps[:ck, m, :L],
                                         k_ap[:, c * 128:c * 128 + ck], q_ap[:],
                                         start=True, stop=True)
                        p_tiles.append((c, ck, p_sb, m))
                    if npair == 2:
                        nc.scalar.activation(p_sb[:, :, :L], t_ps[:, :, :L],
                                             mybir.ActivationFunctionType.Exp,
                                             scale=scale)
                    else:
                        nc.scalar.activation(p_sb[:, 0, :L], t_ps[:, 0, :L],
                                             mybir.ActivationFunctionType.Exp,
                                             scale=scale)
                if pending is not None:
                    emit_av_s(*pending)
                pending = (j, L, nkc, o0, c0, p_tiles)
            emit_av_s(*pending, last=True)
            # one contiguous den store (single descriptor) on the scalar
            # queue, free after the last exp
            nc.scalar.dma_start(den_d[:], den_sb[:])

    nc.compile()
    _program_cache[key] = nc
    return nc


def _reference_host(Q, K, V, num_batch, batch_seg):
    """Pure-NumPy fallback for input shapes outside the tuned envelope."""
    dot = Q.astype(np.float64) @ K.T.astype(np.float64)
    A = np.exp((dot - dot.max()) / np.sqrt(np.float64(Q.shape[-1])))
    if num_batch > 1:
        A = np.where(batch_seg[None, :] == batch_seg[:, None], A, 0.0)
    return ((A / (A.sum(-1, keepdims=True) + EPS)) @ V.astype(np.float64)
            ).astype(np.float32)


def kernel(Q, K, V, num_batch, batch_seg):
    Q = np.asarray(Q, dtype=np.float32)
    K = np.asarray(K, dtype=np.float32)
    V = np.asarray(V, dtype=np.float32)
    batch_seg = np.asarray(batch_seg)
    N = Q.shape[0]
    nb = int(num_batch)

    counts = np.bincount(batch_seg.astype(np.int64), minlength=max(nb, 1))
    if nb < 2 or (counts.max() if nb else N) > 512:
        return _reference_host(Q, K, V, nb, batch_seg)

    # row indices per segment (robust to unsorted batch_seg)
    row_order = np.argsort(batch_seg, kind="stable")
    starts = np.zeros(nb + 1, dtype=np.int64)
    np.cumsum(counts, out=starts[1:])

    # rank segments by length desc, group into slots of 8.  Slot order:
    # ascending by group max, except the largest group is placed 3rd and
    # the smallest LAST — early slots eat the cold-clock window, and the
    # smallest slot's output chain ends the kernel.
    order = np.argsort(-counts, kind="stable")
    n_slots = (nb + N_CORES - 1) // N_CORES
    groups = []  # (Lmax, members) ascending by Lmax
    for j in range(n_slots):
        grp = order[(n_slots - 1 - j) * N_CORES:(n_slots - j) * N_CORES]
        groups.append((max(1, int(counts[grp].max())), grp))
    perm = list(range(n_slots))
    if n_slots >= 2:
        perm = perm[1:] + perm[:1]  # smallest last, 2nd-smallest first
    slot_lens = []
    assign = {}  # (core, slot) -> seg id
    for j, gi in enumerate(perm):
        Lmax, grp = groups[gi]
        slot_lens.append(Lmax)
        for c, seg in enumerate(grp):
            assign[(c, j)] = int(seg)

    offs = np.concatenate([[0], np.cumsum(slot_lens)]).astype(int)
    nkcs = [(L + 127) // 128 for L in slot_lens]
    choffs = np.concatenate([[0], np.cumsum(nkcs)]).astype(int)
    R = int(offs[-1])
    C = int(choffs[-1])

    nc = _build_program(tuple(slot_lens))

    in_maps = []
    for core in range(N_CORES):
        Qp = np.zeros((R, D), np.float32)
        Kp = np.zeros((R, D), np.float32)
        Vp = np.zeros((C * 128, D), np.float32)
        for j in range(n_slots):
            seg = assign.get((core, j))
            if seg is None:
                continue
            b0, b1 = starts[seg], starts[seg + 1]
            ln = int(b1 - b0)
            if ln == 0:
                continue
            ridx = row_order[b0:b1]
            o0 = int(offs[j])
            Qp[o0:o0 + ln] = Q[ridx]
            Kp[o0:o0 + ln] = K[ridx]
            v0 = int(choffs[j]) * 128
            Vp[v0:v0 + ln] = V[ridx]
        vh = np.ascontiguousarray(
            Vp.reshape(C, 128, D).transpose(1, 0, 2)
        ).reshape(128, C * D).astype(np.float16)
        in_maps.append({
            "q": np.ascontiguousarray(Qp.T).astype(np.float16),
            "k": np.ascontiguousarray(Kp.T).astype(np.float16),
            "vx": vh,
        })

    global _last_in_maps
    _last_in_maps = in_maps
    res = bass_utils.run_bass_kernel_spmd(nc, in_maps,
                                          core_ids=list(range(N_CORES)))

    out = np.empty((N, D), np.float32)
    for (core, j), seg in assign.items():
        b0, b1 = starts[seg], starts[seg + 1]
        ln = int(b1 - b0)
        if ln == 0:
            continue
        o0 = int(offs[j])
        L = slot_lens[j]
        otT = res.results[core]["o"][:, o0:o0 + ln].astype(np.float32)
        den_raw = res.results[core]["den"][0, o0:o0 + ln].astype(np.float64)
        # padded keys (zero K) contribute exp(0)=1 each to the device den
        den = den_raw - float(L - ln) + EPS
        out[row_order[b0:b1]] = (otT / den[None, :]).T.astype(np.float32)
    return out
